# revision 8
# baseline (speedup 1.0000x reference)
"""ARD Bayesian Linear layer on 8 Trainium2 NeuronCores.

Strategy:
  - Batch-shard x / dropout_u 8 ways (1024 rows per core).
  - Shard weight_mu/rho/noise and ard_alpha/beta along in_features 8 ways
    (512 cols per core). Each core samples its slice of the weight,
    folds in ard_scale/keep, casts to bf16, transposes it on-chip
    (xbar DMA transpose) and AllGathers the transposed weight in 4
    out-feature groups so the matmul can start while later groups are
    still in flight.
  - y = (x * mask)_bf16 @ W'^T, accumulated in fp32 PSUM. PSUM is seeded
    with the bias via a K=1 ones x bias matmul; two N=512 matmuls share
    each stationary LDWEIGHTS so the weight-load hides.
  - KL partial sums reduced on-device per core ([1,12] vector of raw
    sums), finished on the host during unshard.
"""

import sys

if "/opt/trn_rl_repo" not in sys.path:
    sys.path.insert(0, "/opt/trn_rl_repo")

import numpy as np

import concourse.mybir as mybir
import concourse.tile as tile
from concourse import bacc
from concourse.bass_utils import run_bass_kernel_spmd

NCORES = 8
P = 128
IN_F = 4096
OUT_F = 4096
BATCH = 8192
BSH = BATCH // NCORES  # 1024 batch rows per core
ISH = IN_F // NCORES  # 512 in_features per core
KEEP = 0.9  # 1 - dropout_rate
N_OTILE = OUT_F // P  # 32 weight o-tiles per core
N_OG = 4  # AllGather out-feature groups (1024 cols each)
OGW = OUT_F // N_OG  # 1024
N_BT = BSH // P  # 8 batch tiles per core
N_ISUB = IN_F // P  # 32 contraction subtiles

f32 = mybir.dt.float32
bf16 = mybir.dt.bfloat16
Act = mybir.ActivationFunctionType
Alu = mybir.AluOpType

_CACHED_NC = None
LAST_RESULTS = None  # test harness can read exec_time_ns from here


def _ensure_axon_hooks_shim():
    """This image's ``antenv`` lacks ``axon_hooks``; bass_utils imports it
    unconditionally when tracing is requested. Install a functional shim
    (NTFF profiling via ctypes into libaxon_pjrt.so, mirroring
    trn_agent_boot) so BASS_TRACE=1 works instead of crashing."""
    import types

    try:
        import antenv  # noqa: F401
    except ImportError:
        return
    if "antenv.axon_hooks" in sys.modules:
        return
    try:
        from antenv import axon_hooks  # noqa: F401

        return
    except ImportError:
        pass
    mod = types.ModuleType("antenv.axon_hooks")
    _state = {"hook": None}

    def set_axon_ntff_profile_hook(h):
        _state["hook"] = h

    def get_axon_ntff_profile_hook():
        if _state["hook"] is None:
            try:
                from trn_agent_boot.trn_boot import _ntff_profile_via_ctypes

                _state["hook"] = _ntff_profile_via_ctypes("/opt/axon/libaxon_pjrt.so")
            except Exception:
                _state["hook"] = None
        return _state["hook"]

    mod.set_axon_ntff_profile_hook = set_axon_ntff_profile_hook
    mod.get_axon_ntff_profile_hook = get_axon_ntff_profile_hook
    sys.modules["antenv.axon_hooks"] = mod
    import antenv as _antenv

    _antenv.axon_hooks = mod


_ensure_axon_hooks_shim()


def _softplus(nc, pool, x_ap, shape, tagp):
    """softplus(x) = max(x,0) + ln(1 + exp(-|x|)), exact-ish via Exp/Ln LUTs."""
    ax = pool.tile(shape, f32, tag=tagp + "ax", name=tagp + "ax")
    nc.scalar.activation(ax[:], x_ap, Act.Abs)
    z = pool.tile(shape, f32, tag=tagp + "z", name=tagp + "z")
    nc.scalar.activation(z[:], ax[:], Act.Exp, scale=-1.0)
    l1p = pool.tile(shape, f32, tag=tagp + "l1p", name=tagp + "l1p")
    nc.scalar.activation(l1p[:], z[:], Act.Ln, bias=1.0)
    mx = pool.tile(shape, f32, tag=tagp + "mx", name=tagp + "mx")
    nc.vector.tensor_scalar_max(mx[:], x_ap, 0.0)
    sp = pool.tile(shape, f32, tag=tagp + "sp", name=tagp + "sp")
    nc.vector.tensor_tensor(sp[:], mx[:], l1p[:], Alu.add)
    return sp


def build():
    nc = bacc.Bacc(None, target_bir_lowering=False, num_devices=NCORES)

    x_in = nc.declare_dram_parameter("x", [BSH, IN_F], f32, isOutput=False)
    u_in = nc.declare_dram_parameter("dropout_u", [BSH, IN_F], f32, isOutput=False)
    wmu_in = nc.declare_dram_parameter("weight_mu", [OUT_F, ISH], f32, isOutput=False)
    wrho_in = nc.declare_dram_parameter("weight_rho", [OUT_F, ISH], f32, isOutput=False)
    wnoi_in = nc.declare_dram_parameter(
        "weight_noise", [OUT_F, ISH], f32, isOutput=False
    )
    bmu_in = nc.declare_dram_parameter("bias_mu", [OUT_F], f32, isOutput=False)
    brho_in = nc.declare_dram_parameter("bias_rho", [OUT_F], f32, isOutput=False)
    bnoi_in = nc.declare_dram_parameter("bias_noise", [OUT_F], f32, isOutput=False)
    aa_in = nc.declare_dram_parameter("ard_alpha", [ISH], f32, isOutput=False)
    ab_in = nc.declare_dram_parameter("ard_beta", [ISH], f32, isOutput=False)

    out = nc.declare_dram_parameter("out", [BSH, OUT_F], f32, isOutput=True)
    sums_out = nc.declare_dram_parameter("sums", [1, 12], f32, isOutput=True)

    # internal DRAM: per-out-group collective buffers
    ccin = [nc.dram_tensor(f"ccin{g}", [ISH, OGW], bf16) for g in range(N_OG)]
    gath = [
        nc.dram_tensor(f"gath{g}", [NCORES * ISH, OGW], bf16, addr_space="Shared")
        for g in range(N_OG)
    ]
    asc = nc.dram_tensor("ard_scale_scratch", [ISH], f32)
    sig_sc = nc.dram_tensor("sigma_scratch", [OUT_F, ISH], bf16)
    bsc = nc.dram_tensor("bias_scratch", [OUT_F], f32)

    with tile.TileContext(nc) as tc:
        with (
            tc.tile_pool(name="const", bufs=1) as const,
            tc.tile_pool(name="xdTp", bufs=1) as xdTp,
            tc.tile_pool(name="xdp", bufs=3) as xdp,
            tc.tile_pool(name="accp", bufs=1) as accp,
        ):
            xdT = xdTp.tile([P, N_ISUB, BSH], bf16)  # 8 MB, resident

            klsb = accp.tile([P, 12], f32)
            nc.vector.memset(klsb[:], 0.0)
            accW_ln = accp.tile([P, 2 * N_OTILE], f32)
            accW_rr = accp.tile([P, 2 * N_OTILE], f32)
            accW_mu2 = accp.tile([P, 2 * N_OTILE], f32)
            ones1 = accp.tile([1, P], f32)
            nc.vector.memset(ones1[:], 1.0)

            with (
                tc.tile_pool(name="wgen", bufs=2) as wgen,
                tc.tile_pool(name="wTp", bufs=2) as wTp,
            ):
                # ---------------- phase A: small vectors ----------------
                AF = ISH // P  # 4
                ard_a = wgen.tile([P, AF], f32, tag="arda")
                nc.scalar.dma_start(ard_a[:], aa_in.ap().rearrange("(p f) -> p f", p=P))
                ard_b = wgen.tile([P, AF], f32, tag="ardb")
                nc.scalar.dma_start(ard_b[:], ab_in.ap().rearrange("(p f) -> p f", p=P))
                sa = _softplus(nc, wgen, ard_a[:], [P, AF], "sa")
                sb_ = _softplus(nc, wgen, ard_b[:], [P, AF], "sb")
                # kl cols 3,4,5: sum(sa+sb), sum(ln sa), sum(ln sb)
                tsum = wgen.tile([P, AF], f32, tag="tsum")
                nc.vector.tensor_tensor(tsum[:], sa[:], sb_[:], Alu.add)
                nc.vector.reduce_sum(klsb[:, 3:4], tsum[:], axis=mybir.AxisListType.X)
                scrA = wgen.tile([P, AF], f32, tag="scrA")
                nc.scalar.activation(scrA[:], sa[:], Act.Ln, accum_out=klsb[:, 4:5])
                scrB = wgen.tile([P, AF], f32, tag="scrB")
                nc.scalar.activation(scrB[:], sb_[:], Act.Ln, accum_out=klsb[:, 5:6])
                # ard_scale = sa*sb/keep -> DRAM -> broadcast [P, ISH]
                ascl = wgen.tile([P, AF], f32, tag="ascl")
                nc.vector.tensor_tensor(ascl[:], sa[:], sb_[:], Alu.mult)
                nc.vector.tensor_scalar_mul(ascl[:], ascl[:], 1.0 / KEEP)
                nc.scalar.dma_start(asc.ap().rearrange("(p f) -> p f", p=P), ascl[:])
                ard_rep = const.tile([P, ISH], f32)
                nc.gpsimd.dma_start(
                    ard_rep[:], asc.ap()[None, :].to_broadcast((P, ISH))
                )

                # bias sample + bias KL (cols 6,7,8)
                BF = OUT_F // P  # 32
                bmu = wgen.tile([P, BF], f32, tag="bmu")
                nc.scalar.dma_start(bmu[:], bmu_in.ap().rearrange("(p f) -> p f", p=P))
                brho = wgen.tile([P, BF], f32, tag="brho")
                nc.scalar.dma_start(
                    brho[:], brho_in.ap().rearrange("(p f) -> p f", p=P)
                )
                bnoi = wgen.tile([P, BF], f32, tag="bnoi")
                nc.scalar.dma_start(
                    bnoi[:], bnoi_in.ap().rearrange("(p f) -> p f", p=P)
                )
                sigb = _softplus(nc, wgen, brho[:], [P, BF], "sgb")
                scrC = wgen.tile([P, BF], f32, tag="scrC")
                nc.scalar.activation(scrC[:], sigb[:], Act.Ln, accum_out=klsb[:, 6:7])
                rb = wgen.tile([P, BF], f32, tag="rb")
                nc.vector.reciprocal(rb[:], sigb[:])
                scrD = wgen.tile([P, BF], f32, tag="scrD")
                nc.scalar.activation(scrD[:], rb[:], Act.Square, accum_out=klsb[:, 7:8])
                scrE = wgen.tile([P, BF], f32, tag="scrE")
                nc.scalar.activation(
                    scrE[:], bmu[:], Act.Square, accum_out=klsb[:, 8:9]
                )
                bsamp = wgen.tile([P, BF], f32, tag="bsamp")
                nc.vector.tensor_tensor(bsamp[:], sigb[:], bnoi[:], Alu.mult)
                nc.vector.tensor_tensor(bsamp[:], bsamp[:], bmu[:], Alu.add)
                nc.scalar.dma_start(bsc.ap().rearrange("(p f) -> p f", p=P), bsamp[:])

                # ------- phase B: weight gen + transpose + per-group AG ---
                wTg = None
                for t in range(N_OTILE):
                    if t % (N_OTILE // N_OG) == 0:
                        g = t // (N_OTILE // N_OG)
                        wTg = wTp.tile(
                            [P, ISH // P, OGW], bf16, tag="wTg", name=f"wTg{g}"
                        )
                    sl = slice(t * P, (t + 1) * P)
                    lsl = slice((t % (N_OTILE // N_OG)) * P,
                                (t % (N_OTILE // N_OG) + 1) * P)
                    mu_t = wgen.tile([P, ISH], f32, tag="mu")
                    nc.sync.dma_start(mu_t[:], wmu_in[sl, :])
                    rho_t = wgen.tile([P, ISH], f32, tag="rho")
                    nc.sync.dma_start(rho_t[:], wrho_in[sl, :])
                    noi_t = wgen.tile([P, ISH], f32, tag="noi")
                    nc.sync.dma_start(noi_t[:], wnoi_in[sl, :])

                    # softplus with max(x,0) on ACT (Relu) to balance engines
                    axp = wgen.tile([P, ISH], f32, tag="sgax")
                    nc.scalar.activation(axp[:], rho_t[:], Act.Abs)
                    zp = wgen.tile([P, ISH], f32, tag="sgz")
                    nc.scalar.activation(zp[:], axp[:], Act.Exp, scale=-1.0)
                    l1pp = wgen.tile([P, ISH], f32, tag="sgl1p")
                    nc.scalar.activation(l1pp[:], zp[:], Act.Ln, bias=1.0)
                    mxp = wgen.tile([P, ISH], f32, tag="sgmx")
                    nc.scalar.activation(mxp[:], rho_t[:], Act.Relu)
                    sig = wgen.tile([P, ISH], f32, tag="sgsp")
                    nc.vector.tensor_tensor(sig[:], mxp[:], l1pp[:], Alu.add)
                    # spill sigma (bf16) for the deferred KL pass
                    sigb16 = wgen.tile([P, ISH], bf16, tag="sgb16")
                    nc.vector.tensor_copy(sigb16[:], sig[:])
                    nc.sync.dma_start(sig_sc[sl, :], sigb16[:])
                    # W' = (mu + sig*noise) * ard_scale, bf16
                    w_t = wgen.tile([P, ISH], f32, tag="wt")
                    nc.vector.tensor_tensor(w_t[:], sig[:], noi_t[:], Alu.mult)
                    nc.vector.tensor_tensor(w_t[:], w_t[:], mu_t[:], Alu.add)
                    wb_t = wgen.tile([P, ISH], bf16, tag="wb")
                    nc.vector.tensor_tensor(wb_t[:], w_t[:], ard_rep[:], Alu.mult)
                    # xbar transpose into W'^T
                    nc.sync.dma_start(wTg[:, :, lsl], wb_t[:], transpose=True)

                    # end of an out-feature group: ship it + AllGather
                    # (ccin write + trigger both on the gpsimd queue so the
                    #  collective chain never blocks the SP DMA queue)
                    if (t + 1) % (N_OTILE // N_OG) == 0:
                        g = t // (N_OTILE // N_OG)
                        nc.gpsimd.dma_start(
                            ccin[g].ap().rearrange("(s p) o -> p s o", p=P),
                            wTg[:],
                        )
                        nc.gpsimd.collective_compute(
                            "AllGather",
                            Alu.bypass,
                            replica_groups=[list(range(NCORES))],
                            ins=[ccin[g][:, :]],
                            outs=[gath[g][:, :]],
                        )

            # ---------------- phase C: xd = x * mask, bf16, transposed ---
            CW = 512  # i-chunk width for the xd pipeline
            for b in range(N_BT):
                bsl = slice(b * P, (b + 1) * P)
                for c in range(IN_F // CW):
                    csl = slice(c * CW, (c + 1) * CW)
                    x_c = xdp.tile([P, CW], f32, tag="x")
                    nc.sync.dma_start(x_c[:], x_in[bsl, csl])
                    u_c = xdp.tile([P, CW], f32, tag="u")
                    nc.sync.dma_start(u_c[:], u_in[bsl, csl])
                    nc.vector.tensor_scalar(u_c[:], u_c[:], KEEP, None, Alu.is_lt)
                    xdb = xdp.tile([P, CW], bf16, tag="xdb")
                    nc.vector.tensor_tensor(xdb[:], x_c[:], u_c[:], Alu.mult)
                    nc.sync.dma_start(
                        xdT[:, c * (CW // P) : (c + 1) * (CW // P), bsl],
                        xdb[:],
                        transpose=True,
                    )

            # ---------------- phase D: matmul ----------------
            with (
                tc.tile_pool(name="gp", bufs=5) as gp,
                tc.tile_pool(name="yp", bufs=2) as yp,
                tc.tile_pool(name="bs1", bufs=1) as bs1,
                tc.tile_pool(name="kle", bufs=1) as kle,
                tc.tile_pool(name="psmm", bufs=8, space="PSUM") as psmm,
            ):
                bias1 = bs1.tile([1, OUT_F], f32)
                nc.scalar.dma_start(bias1[:], bsc.ap()[None, :])
                HI = N_ISUB // 2  # 16 i-subtiles per quarter tile
                for g in range(N_OG):
                    # quarter tiles: [o-half h][i-half q] of the gathered group
                    gts = [[None, None], [None, None]]
                    for h in range(2):
                        for q in range(2):
                            gt = gp.tile(
                                [P, HI, 512], bf16, tag="g", name=f"g{g}_{h}_{q}"
                            )
                            nc.gpsimd.dma_start(
                                gt[:],
                                gath[g][
                                    q * HI * P : (q + 1) * HI * P,
                                    h * 512 : (h + 1) * 512,
                                ].rearrange("(q p) o -> p q o", p=P),
                            )
                            gts[h][q] = gt
                    for bh in range(2):
                        pss = [
                            [
                                psmm.tile(
                                    [P, 512], f32, tag="mm", name=f"ps{g}_{bh}_{b4}_{h}"
                                )
                                for h in range(2)
                            ]
                            for b4 in range(4)
                        ]
                        # seed all banks with the bias (K=1 ones x bias row)
                        for b4 in range(4):
                            for h in range(2):
                                ob = g * OGW + h * 512
                                nc.tensor.matmul(
                                    pss[b4][h][:],
                                    ones1[:],
                                    bias1[0:1, ob : ob + 512],
                                    start=True,
                                    stop=False,
                                )
                        for isub in range(N_ISUB):
                            q, iq = divmod(isub, HI)
                            for b4 in range(4):
                                b = bh * 4 + b4
                                lhs = xdT[:, isub, b * P : (b + 1) * P]
                                last = isub == N_ISUB - 1
                                nc.tensor.matmul(
                                    pss[b4][0][:],
                                    lhs,
                                    gts[0][q][:, iq, :],
                                    start=False,
                                    stop=last,
                                )
                                nc.tensor.matmul(
                                    pss[b4][1][:],
                                    lhs,
                                    gts[1][q][:, iq, :],
                                    start=False,
                                    stop=last,
                                )
                        for b4 in range(4):
                            b = bh * 4 + b4
                            for h in range(2):
                                osl = slice(g * OGW + h * 512, g * OGW + (h + 1) * 512)
                                y_sb = yp.tile([P, 512], f32, tag="y")
                                nc.vector.tensor_copy(y_sb[:], pss[b4][h][:])
                                nc.sync.dma_start(
                                    out[b * P : (b + 1) * P, osl], y_sb[:]
                                )

                # ---- deferred weight KL: runs on idle ACT/DVE during matmuls
                # accumulates into 64 half-tile columns
                for t2 in range(2 * N_OTILE):
                    t, hh = divmod(t2, 2)
                    sl = slice(t * P, (t + 1) * P)
                    hsl = slice(hh * (ISH // 2), (hh + 1) * (ISH // 2))
                    HS = ISH // 2
                    sgb = kle.tile([P, HS], bf16, tag="ksg")
                    nc.scalar.dma_start(sgb[:], sig_sc[sl, hsl])
                    mu2 = kle.tile([P, HS], f32, tag="kmu")
                    nc.scalar.dma_start(mu2[:], wmu_in[sl, hsl])
                    kscr = kle.tile([P, HS], f32, tag="kscr")
                    nc.scalar.activation(
                        kscr[:], sgb[:], Act.Ln, accum_out=accW_ln[:, t2 : t2 + 1]
                    )
                    sgf = kle.tile([P, HS], f32, tag="ksgf")
                    nc.vector.tensor_copy(sgf[:], sgb[:])
                    rr = kle.tile([P, HS], f32, tag="krr")
                    nc.vector.reciprocal_approx_fast(rr[:], sgf[:])
                    kscr2 = kle.tile([P, HS], f32, tag="kscr")
                    nc.scalar.activation(
                        kscr2[:], rr[:], Act.Square, accum_out=accW_rr[:, t2 : t2 + 1]
                    )
                    kscr3 = kle.tile([P, HS], f32, tag="kscr")
                    nc.scalar.activation(
                        kscr3[:], mu2[:], Act.Square, accum_out=accW_mu2[:, t2 : t2 + 1]
                    )

            # ---------------- phase E: KL finish ----------------
            with tc.tile_pool(name="pskl", bufs=1, space="PSUM") as pskl:
                nc.vector.reduce_sum(
                    klsb[:, 0:1], accW_ln[:], axis=mybir.AxisListType.X
                )
                nc.vector.reduce_sum(
                    klsb[:, 1:2], accW_rr[:], axis=mybir.AxisListType.X
                )
                nc.vector.reduce_sum(
                    klsb[:, 2:3], accW_mu2[:], axis=mybir.AxisListType.X
                )
                ones_t = accp.tile([P, 1], f32)
                nc.vector.memset(ones_t[:], 1.0)
                pk = pskl.tile([P, 512], f32)
                nc.tensor.matmul(
                    pk[0:1, 0:12], ones_t[:], klsb[:], start=True, stop=True
                )
                s_sb = accp.tile([1, 12], f32)
                nc.vector.tensor_copy(s_sb[:], pk[0:1, 0:12])
                nc.sync.dma_start(sums_out[:, :], s_sb[:])

    nc.compile()
    return nc


def _get_nc():
    global _CACHED_NC
    if _CACHED_NC is None:
        _CACHED_NC = build()
    return _CACHED_NC


def kernel(
    x,
    weight_mu,
    weight_rho,
    bias_mu,
    bias_rho,
    ard_alpha,
    ard_beta,
    weight_noise,
    bias_noise,
    dropout_u,
):
    global LAST_RESULTS
    x = np.asarray(x, np.float32)
    weight_mu = np.asarray(weight_mu, np.float32)
    weight_rho = np.asarray(weight_rho, np.float32)
    bias_mu = np.asarray(bias_mu, np.float32)
    bias_rho = np.asarray(bias_rho, np.float32)
    ard_alpha = np.asarray(ard_alpha, np.float32)
    ard_beta = np.asarray(ard_beta, np.float32)
    weight_noise = np.asarray(weight_noise, np.float32)
    bias_noise = np.asarray(bias_noise, np.float32)
    dropout_u = np.asarray(dropout_u, np.float32)

    nc = _get_nc()
    in_maps = []
    for r in range(NCORES):
        bsl = slice(r * BSH, (r + 1) * BSH)
        isl = slice(r * ISH, (r + 1) * ISH)
        in_maps.append(
            {
                "x": np.ascontiguousarray(x[bsl]),
                "dropout_u": np.ascontiguousarray(dropout_u[bsl]),
                "weight_mu": np.ascontiguousarray(weight_mu[:, isl]),
                "weight_rho": np.ascontiguousarray(weight_rho[:, isl]),
                "weight_noise": np.ascontiguousarray(weight_noise[:, isl]),
                "bias_mu": bias_mu,
                "bias_rho": bias_rho,
                "bias_noise": bias_noise,
                "ard_alpha": np.ascontiguousarray(ard_alpha[isl]),
                "ard_beta": np.ascontiguousarray(ard_beta[isl]),
            }
        )

    res = run_bass_kernel_spmd(nc, in_maps, core_ids=list(range(NCORES)))
    LAST_RESULTS = res
    outs = res.results

    output = np.concatenate([outs[r]["out"] for r in range(NCORES)], axis=0)

    s = np.stack([outs[r]["sums"][0].astype(np.float64) for r in range(NCORES)])
    weight_kl = 0.5 * (2.0 * s[:, 0].sum() + s[:, 1].sum() + s[:, 2].sum()) - 0.5 * (
        OUT_F * IN_F
    )
    ard_kl = (s[:, 3] - s[:, 4] - s[:, 5]).sum()
    bias_kl = 0.5 * (2.0 * s[0, 6] + s[0, 7] + s[0, 8]) - 0.5 * OUT_F
    kl = np.float32(weight_kl + ard_kl + bias_kl)

    return output, kl


# revision 10
# speedup vs baseline: 1.2209x; 1.2209x over previous
"""ARD Bayesian Linear layer on 8 Trainium2 NeuronCores.

Strategy:
  - Batch-shard x / dropout_u 8 ways (1024 rows per core).
  - Shard weight_mu/rho/noise and ard_alpha/beta along in_features 8 ways
    (512 cols per core). Each core samples its slice of the weight,
    folds in ard_scale/keep, casts to bf16, transposes it on-chip
    (xbar DMA transpose) and AllGathers the transposed weight in 4
    out-feature groups so the matmul can start while later groups are
    still in flight.
  - y = (x * mask)_bf16 @ W'^T, accumulated in fp32 PSUM. PSUM is seeded
    with the bias via a K=1 ones x bias matmul; two N=512 matmuls share
    each stationary LDWEIGHTS so the weight-load hides.
  - KL partial sums reduced on-device per core ([1,12] vector of raw
    sums), finished on the host during unshard.
"""

import sys

if "/opt/trn_rl_repo" not in sys.path:
    sys.path.insert(0, "/opt/trn_rl_repo")

import numpy as np

import concourse.mybir as mybir
import concourse.tile as tile
from concourse import bacc
from concourse.bass_utils import run_bass_kernel_spmd

NCORES = 8
P = 128
IN_F = 4096
OUT_F = 4096
BATCH = 8192
BSH = BATCH // NCORES  # 1024 batch rows per core
ISH = IN_F // NCORES  # 512 in_features per core
KEEP = 0.9  # 1 - dropout_rate
N_OTILE = OUT_F // P  # 32 weight o-tiles per core
N_OG = 4  # AllGather out-feature groups (1024 cols each)
OGW = OUT_F // N_OG  # 1024
N_BT = BSH // P  # 8 batch tiles per core
N_ISUB = IN_F // P  # 32 contraction subtiles

f32 = mybir.dt.float32
bf16 = mybir.dt.bfloat16
Act = mybir.ActivationFunctionType
Alu = mybir.AluOpType

_CACHED_NC = None
LAST_RESULTS = None  # test harness can read exec_time_ns from here


def _ensure_axon_hooks_shim():
    """This image's ``antenv`` lacks ``axon_hooks``; bass_utils imports it
    unconditionally when tracing is requested. Install a functional shim
    (NTFF profiling via ctypes into libaxon_pjrt.so, mirroring
    trn_agent_boot) so BASS_TRACE=1 works instead of crashing."""
    import types

    try:
        import antenv  # noqa: F401
    except ImportError:
        return
    if "antenv.axon_hooks" in sys.modules:
        return
    try:
        from antenv import axon_hooks  # noqa: F401

        return
    except ImportError:
        pass
    mod = types.ModuleType("antenv.axon_hooks")
    _state = {"hook": None}

    def set_axon_ntff_profile_hook(h):
        _state["hook"] = h

    def get_axon_ntff_profile_hook():
        if _state["hook"] is None:
            try:
                from trn_agent_boot.trn_boot import _ntff_profile_via_ctypes

                _state["hook"] = _ntff_profile_via_ctypes("/opt/axon/libaxon_pjrt.so")
            except Exception:
                _state["hook"] = None
        return _state["hook"]

    mod.set_axon_ntff_profile_hook = set_axon_ntff_profile_hook
    mod.get_axon_ntff_profile_hook = get_axon_ntff_profile_hook
    sys.modules["antenv.axon_hooks"] = mod
    import antenv as _antenv

    _antenv.axon_hooks = mod


_ensure_axon_hooks_shim()


def _softplus(nc, pool, x_ap, shape, tagp):
    """softplus(x) = max(x,0) + ln(1 + exp(-|x|)), exact-ish via Exp/Ln LUTs."""
    ax = pool.tile(shape, f32, tag=tagp + "ax", name=tagp + "ax")
    nc.scalar.activation(ax[:], x_ap, Act.Abs)
    z = pool.tile(shape, f32, tag=tagp + "z", name=tagp + "z")
    nc.scalar.activation(z[:], ax[:], Act.Exp, scale=-1.0)
    l1p = pool.tile(shape, f32, tag=tagp + "l1p", name=tagp + "l1p")
    nc.scalar.activation(l1p[:], z[:], Act.Ln, bias=1.0)
    mx = pool.tile(shape, f32, tag=tagp + "mx", name=tagp + "mx")
    nc.vector.tensor_scalar_max(mx[:], x_ap, 0.0)
    sp = pool.tile(shape, f32, tag=tagp + "sp", name=tagp + "sp")
    nc.vector.tensor_tensor(sp[:], mx[:], l1p[:], Alu.add)
    return sp


def build():
    nc = bacc.Bacc(None, target_bir_lowering=False, num_devices=NCORES)

    x_in = nc.declare_dram_parameter("x", [BSH, IN_F], f32, isOutput=False)
    u_in = nc.declare_dram_parameter("dropout_u", [BSH, IN_F], f32, isOutput=False)
    wmu_in = nc.declare_dram_parameter("weight_mu", [OUT_F, ISH], f32, isOutput=False)
    wrho_in = nc.declare_dram_parameter("weight_rho", [OUT_F, ISH], f32, isOutput=False)
    wnoi_in = nc.declare_dram_parameter(
        "weight_noise", [OUT_F, ISH], f32, isOutput=False
    )
    bmu_in = nc.declare_dram_parameter("bias_mu", [OUT_F], f32, isOutput=False)
    brho_in = nc.declare_dram_parameter("bias_rho", [OUT_F], f32, isOutput=False)
    bnoi_in = nc.declare_dram_parameter("bias_noise", [OUT_F], f32, isOutput=False)
    aa_in = nc.declare_dram_parameter("ard_alpha", [ISH], f32, isOutput=False)
    ab_in = nc.declare_dram_parameter("ard_beta", [ISH], f32, isOutput=False)

    out = nc.declare_dram_parameter("out", [BSH, OUT_F], f32, isOutput=True)
    sums_out = nc.declare_dram_parameter("sums", [1, 12], f32, isOutput=True)

    # internal DRAM: per-out-group collective buffers
    ccin = [nc.dram_tensor(f"ccin{g}", [ISH, OGW], bf16) for g in range(N_OG)]
    gath = [
        nc.dram_tensor(f"gath{g}", [NCORES * ISH, OGW], bf16, addr_space="Shared")
        for g in range(N_OG)
    ]
    asc = nc.dram_tensor("ard_scale_scratch", [ISH], f32)
    bsc = nc.dram_tensor("bias_scratch", [OUT_F], bf16)

    with tile.TileContext(nc) as tc:
        with (
            tc.tile_pool(name="const", bufs=1) as const,
            tc.tile_pool(name="xdTp", bufs=1) as xdTp,
            tc.tile_pool(name="xdp", bufs=2) as xdp,
            tc.tile_pool(name="accp", bufs=1) as accp,
        ):
            xdT = xdTp.tile([P, N_ISUB, BSH], bf16)  # 8 MB, resident

            klsb = accp.tile([P, 12], f32)
            nc.vector.memset(klsb[:], 0.0)
            accW_ln = accp.tile([P, N_OTILE], f32)
            accW_rr = accp.tile([P, N_OTILE], f32)
            accW_mu2 = accp.tile([P, N_OTILE], f32)
            ones1 = accp.tile([1, P], bf16)
            nc.vector.memset(ones1[:], 1.0)

            with (
                tc.tile_pool(name="wgen", bufs=2) as wgen,
                tc.tile_pool(name="wTp", bufs=2) as wTp,
            ):
                # ---------------- phase A: small vectors ----------------
                AF = ISH // P  # 4
                ard_a = wgen.tile([P, AF], f32, tag="arda")
                nc.scalar.dma_start(ard_a[:], aa_in.ap().rearrange("(p f) -> p f", p=P))
                ard_b = wgen.tile([P, AF], f32, tag="ardb")
                nc.scalar.dma_start(ard_b[:], ab_in.ap().rearrange("(p f) -> p f", p=P))
                sa = _softplus(nc, wgen, ard_a[:], [P, AF], "sa")
                sb_ = _softplus(nc, wgen, ard_b[:], [P, AF], "sb")
                # kl cols 3,4,5: sum(sa+sb), sum(ln sa), sum(ln sb)
                tsum = wgen.tile([P, AF], f32, tag="tsum")
                nc.vector.tensor_tensor(tsum[:], sa[:], sb_[:], Alu.add)
                nc.vector.reduce_sum(klsb[:, 3:4], tsum[:], axis=mybir.AxisListType.X)
                scrA = wgen.tile([P, AF], f32, tag="scrA")
                nc.scalar.activation(scrA[:], sa[:], Act.Ln, accum_out=klsb[:, 4:5])
                scrB = wgen.tile([P, AF], f32, tag="scrB")
                nc.scalar.activation(scrB[:], sb_[:], Act.Ln, accum_out=klsb[:, 5:6])
                # ard_scale = sa*sb/keep -> DRAM -> broadcast [P, ISH]
                ascl = wgen.tile([P, AF], f32, tag="ascl")
                nc.vector.tensor_tensor(ascl[:], sa[:], sb_[:], Alu.mult)
                nc.vector.tensor_scalar_mul(ascl[:], ascl[:], 1.0 / KEEP)
                nc.scalar.dma_start(asc.ap().rearrange("(p f) -> p f", p=P), ascl[:])
                ard_rep = const.tile([P, ISH], f32)
                nc.gpsimd.dma_start(
                    ard_rep[:], asc.ap()[None, :].to_broadcast((P, ISH))
                )

                # bias sample + bias KL (cols 6,7,8)
                BF = OUT_F // P  # 32
                bmu = wgen.tile([P, BF], f32, tag="bmu")
                nc.scalar.dma_start(bmu[:], bmu_in.ap().rearrange("(p f) -> p f", p=P))
                brho = wgen.tile([P, BF], f32, tag="brho")
                nc.scalar.dma_start(
                    brho[:], brho_in.ap().rearrange("(p f) -> p f", p=P)
                )
                bnoi = wgen.tile([P, BF], f32, tag="bnoi")
                nc.scalar.dma_start(
                    bnoi[:], bnoi_in.ap().rearrange("(p f) -> p f", p=P)
                )
                sigb = _softplus(nc, wgen, brho[:], [P, BF], "sgb")
                scrC = wgen.tile([P, BF], f32, tag="scrC")
                nc.scalar.activation(scrC[:], sigb[:], Act.Ln, accum_out=klsb[:, 6:7])
                rb = wgen.tile([P, BF], f32, tag="rb")
                nc.vector.reciprocal(rb[:], sigb[:])
                scrD = wgen.tile([P, BF], f32, tag="scrD")
                nc.scalar.activation(scrD[:], rb[:], Act.Square, accum_out=klsb[:, 7:8])
                scrE = wgen.tile([P, BF], f32, tag="scrE")
                nc.scalar.activation(
                    scrE[:], bmu[:], Act.Square, accum_out=klsb[:, 8:9]
                )
                bsamp = wgen.tile([P, BF], f32, tag="bsamp")
                nc.vector.tensor_tensor(bsamp[:], sigb[:], bnoi[:], Alu.mult)
                nc.vector.tensor_tensor(bsamp[:], bsamp[:], bmu[:], Alu.add)
                bsampb = wgen.tile([P, BF], bf16, tag="bsampb")
                nc.vector.tensor_copy(bsampb[:], bsamp[:])
                nc.scalar.dma_start(bsc.ap().rearrange("(p f) -> p f", p=P), bsampb[:])

                # ------- phase B: weight gen + transpose + per-group AG ---
                wTg = None
                mu2t = rho2t = noi2t = None
                for t in range(N_OTILE):
                    if t % (N_OTILE // N_OG) == 0:
                        g = t // (N_OTILE // N_OG)
                        wTg = wTp.tile(
                            [P, ISH // P, OGW], bf16, tag="wTg", name=f"wTg{g}"
                        )
                    if t % 2 == 0:
                        # load two o-tiles of each param per DMA
                        dsl = slice(t * P, (t + 2) * P)
                        mu2t = wgen.tile([P, 2, ISH], f32, tag="mu", name=f"mu{t}")
                        nc.sync.dma_start(
                            mu2t[:], wmu_in[dsl, :].rearrange("(a p) i -> p a i", p=P)
                        )
                        rho2t = wgen.tile([P, 2, ISH], f32, tag="rho", name=f"rho{t}")
                        nc.sync.dma_start(
                            rho2t[:], wrho_in[dsl, :].rearrange("(a p) i -> p a i", p=P)
                        )
                        noi2t = wgen.tile([P, 2, ISH], f32, tag="noi", name=f"noi{t}")
                        nc.sync.dma_start(
                            noi2t[:], wnoi_in[dsl, :].rearrange("(a p) i -> p a i", p=P)
                        )
                    lsl = slice((t % (N_OTILE // N_OG)) * P,
                                (t % (N_OTILE // N_OG) + 1) * P)
                    mu_t = mu2t[:, t % 2, :]
                    rho_t = rho2t[:, t % 2, :]
                    noi_t = noi2t[:, t % 2, :]

                    # softplus chain (Abs/Exp/Ln + Relu on ACT, add on DVE)
                    axp = wgen.tile([P, ISH], f32, tag="ch", name=f"ax{t}")
                    nc.scalar.activation(axp[:], rho_t, Act.Abs)
                    zp = wgen.tile([P, ISH], f32, tag="ch", name=f"z{t}")
                    nc.scalar.activation(zp[:], axp[:], Act.Exp, scale=-1.0)
                    l1pp = wgen.tile([P, ISH], f32, tag="ch", name=f"l1p{t}")
                    nc.scalar.activation(l1pp[:], zp[:], Act.Ln, bias=1.0)
                    mxp = wgen.tile([P, ISH], f32, tag="mx", name=f"mx{t}")
                    nc.scalar.activation(mxp[:], rho_t, Act.Relu)
                    sig = wgen.tile([P, ISH], f32, tag="sgsp", name=f"sig{t}")
                    nc.vector.tensor_tensor(sig[:], mxp[:], l1pp[:], Alu.add)
                    # W' = (mu + sig*noise) * ard_scale, bf16
                    w_t = wgen.tile([P, ISH], f32, tag="wt", name=f"w{t}")
                    nc.vector.tensor_tensor(w_t[:], sig[:], noi_t, Alu.mult)
                    nc.vector.tensor_tensor(w_t[:], w_t[:], mu_t, Alu.add)
                    wb_t = wgen.tile([P, ISH], bf16, tag="wb", name=f"wb{t}")
                    nc.vector.tensor_tensor(wb_t[:], w_t[:], ard_rep[:], Alu.mult)
                    # xbar transpose into W'^T
                    nc.sync.dma_start(wTg[:, :, lsl], wb_t[:], transpose=True)

                    # end of an out-feature group: ship it + AllGather
                    # (ccin write on the ACT queue right behind the group's
                    #  compute; AG trigger on the gpsimd queue)
                    if (t + 1) % (N_OTILE // N_OG) == 0:
                        g = t // (N_OTILE // N_OG)
                        nc.scalar.dma_start(
                            ccin[g].ap().rearrange("(s p) o -> p s o", p=P),
                            wTg[:],
                        )
                        nc.gpsimd.collective_compute(
                            "AllGather",
                            Alu.bypass,
                            replica_groups=[list(range(NCORES))],
                            ins=[ccin[g][:, :]],
                            outs=[gath[g][:, :]],
                        )

            # ---------------- phase C: xd = x * mask, bf16, transposed ---
            CW = 1024  # i-chunk width for the xd pipeline
            for b in range(N_BT):
                bsl = slice(b * P, (b + 1) * P)
                for c in range(IN_F // CW):
                    csl = slice(c * CW, (c + 1) * CW)
                    x_c = xdp.tile([P, CW], f32, tag="x")
                    nc.sync.dma_start(x_c[:], x_in[bsl, csl])
                    u_c = xdp.tile([P, CW], f32, tag="u")
                    nc.scalar.dma_start(u_c[:], u_in[bsl, csl])
                    nc.vector.tensor_scalar(u_c[:], u_c[:], KEEP, None, Alu.is_lt)
                    xdb = xdp.tile([P, CW], bf16, tag="xdb")
                    nc.vector.tensor_tensor(xdb[:], x_c[:], u_c[:], Alu.mult)
                    nc.sync.dma_start(
                        xdT[:, c * (CW // P) : (c + 1) * (CW // P), bsl],
                        xdb[:],
                        transpose=True,
                    )

            # ---------------- phase D: matmul ----------------
            with (
                tc.tile_pool(name="gp", bufs=6) as gp,
                tc.tile_pool(name="yp", bufs=3) as yp,
                tc.tile_pool(name="bs1", bufs=1) as bs1,
                tc.tile_pool(name="kle", bufs=2) as kle,
                tc.tile_pool(name="psmm", bufs=8, space="PSUM") as psmm,
            ):
                bias1 = bs1.tile([1, OUT_F], bf16)
                nc.scalar.dma_start(bias1[:], bsc.ap()[None, :])
                EI = 8  # i-subtiles per G eighth-tile
                NE = N_ISUB // EI  # 4 eighths
                for g in range(N_OG):
                    for h in range(2):
                        ob = g * OGW + h * 512
                        # 4 i-eighth tiles of this (group, o-half)
                        gts = [None] * NE
                        for e in range(NE):
                            gt = gp.tile(
                                [P, EI, 512], bf16, tag="g", name=f"g{g}_{h}_{e}"
                            )
                            nc.gpsimd.dma_start(
                                gt[:],
                                gath[g][
                                    e * EI * P : (e + 1) * EI * P,
                                    h * 512 : (h + 1) * 512,
                                ].rearrange("(q p) o -> p q o", p=P),
                            )
                            gts[e] = gt
                        for b in range(N_BT):
                            ps = psmm.tile(
                                [P, 512], f32, tag="mm", name=f"ps{g}_{h}_{b}"
                            )
                            # seed with bias (K=1 ones x bias row), then a
                            # same-bank chain of 32 accumulating matmuls
                            nc.tensor.matmul(
                                ps[:],
                                ones1[:],
                                bias1[0:1, ob : ob + 512],
                                start=True,
                                stop=False,
                            )
                            for isub in range(N_ISUB):
                                e, iq = divmod(isub, EI)
                                nc.tensor.matmul(
                                    ps[:],
                                    xdT[:, isub, b * P : (b + 1) * P],
                                    gts[e][:, iq, :],
                                    start=False,
                                    stop=(isub == N_ISUB - 1),
                                )
                            y_sb = yp.tile([P, 512], f32, tag="y")
                            nc.vector.tensor_copy(y_sb[:], ps[:])
                            nc.sync.dma_start(
                                out[b * P : (b + 1) * P, ob : ob + 512], y_sb[:]
                            )

                # ---- deferred weight KL on idle ACT/DVE during matmuls
                # (recompute sigma from a rho reload)
                for t in range(N_OTILE):
                    sl = slice(t * P, (t + 1) * P)
                    krho = kle.tile([P, ISH], f32, tag="krho")
                    nc.scalar.dma_start(krho[:], wrho_in[sl, :])
                    kmu = kle.tile([P, ISH], f32, tag="kmu")
                    nc.scalar.dma_start(kmu[:], wmu_in[sl, :])
                    kax = kle.tile([P, ISH], f32, tag="kch")
                    nc.scalar.activation(kax[:], krho[:], Act.Abs)
                    kz = kle.tile([P, ISH], f32, tag="kch")
                    nc.scalar.activation(kz[:], kax[:], Act.Exp, scale=-1.0)
                    kl1p = kle.tile([P, ISH], f32, tag="kch")
                    nc.scalar.activation(kl1p[:], kz[:], Act.Ln, bias=1.0)
                    kmx = kle.tile([P, ISH], f32, tag="kmx")
                    nc.scalar.activation(kmx[:], krho[:], Act.Relu)
                    ksg = kle.tile([P, ISH], f32, tag="ksg")
                    nc.vector.tensor_tensor(ksg[:], kmx[:], kl1p[:], Alu.add)
                    kscr = kle.tile([P, ISH], f32, tag="kscr")
                    nc.scalar.activation(
                        kscr[:], ksg[:], Act.Ln, accum_out=accW_ln[:, t : t + 1]
                    )
                    krr = kle.tile([P, ISH], f32, tag="krr")
                    nc.vector.reciprocal_approx_fast(krr[:], ksg[:])
                    kscr2 = kle.tile([P, ISH], f32, tag="kscr")
                    nc.scalar.activation(
                        kscr2[:], krr[:], Act.Square, accum_out=accW_rr[:, t : t + 1]
                    )
                    kscr3 = kle.tile([P, ISH], f32, tag="kscr")
                    nc.scalar.activation(
                        kscr3[:], kmu[:], Act.Square, accum_out=accW_mu2[:, t : t + 1]
                    )

            # ---------------- phase E: KL finish ----------------
            with tc.tile_pool(name="pskl", bufs=1, space="PSUM") as pskl:
                nc.vector.reduce_sum(
                    klsb[:, 0:1], accW_ln[:], axis=mybir.AxisListType.X
                )
                nc.vector.reduce_sum(
                    klsb[:, 1:2], accW_rr[:], axis=mybir.AxisListType.X
                )
                nc.vector.reduce_sum(
                    klsb[:, 2:3], accW_mu2[:], axis=mybir.AxisListType.X
                )
                ones_t = accp.tile([P, 1], f32)
                nc.vector.memset(ones_t[:], 1.0)
                pk = pskl.tile([P, 512], f32)
                nc.tensor.matmul(
                    pk[0:1, 0:12], ones_t[:], klsb[:], start=True, stop=True
                )
                s_sb = accp.tile([1, 12], f32)
                nc.vector.tensor_copy(s_sb[:], pk[0:1, 0:12])
                nc.sync.dma_start(sums_out[:, :], s_sb[:])

    nc.compile()
    return nc


def _get_nc():
    global _CACHED_NC
    if _CACHED_NC is None:
        _CACHED_NC = build()
    return _CACHED_NC


def kernel(
    x,
    weight_mu,
    weight_rho,
    bias_mu,
    bias_rho,
    ard_alpha,
    ard_beta,
    weight_noise,
    bias_noise,
    dropout_u,
):
    global LAST_RESULTS
    x = np.asarray(x, np.float32)
    weight_mu = np.asarray(weight_mu, np.float32)
    weight_rho = np.asarray(weight_rho, np.float32)
    bias_mu = np.asarray(bias_mu, np.float32)
    bias_rho = np.asarray(bias_rho, np.float32)
    ard_alpha = np.asarray(ard_alpha, np.float32)
    ard_beta = np.asarray(ard_beta, np.float32)
    weight_noise = np.asarray(weight_noise, np.float32)
    bias_noise = np.asarray(bias_noise, np.float32)
    dropout_u = np.asarray(dropout_u, np.float32)

    nc = _get_nc()
    in_maps = []
    for r in range(NCORES):
        bsl = slice(r * BSH, (r + 1) * BSH)
        isl = slice(r * ISH, (r + 1) * ISH)
        in_maps.append(
            {
                "x": np.ascontiguousarray(x[bsl]),
                "dropout_u": np.ascontiguousarray(dropout_u[bsl]),
                "weight_mu": np.ascontiguousarray(weight_mu[:, isl]),
                "weight_rho": np.ascontiguousarray(weight_rho[:, isl]),
                "weight_noise": np.ascontiguousarray(weight_noise[:, isl]),
                "bias_mu": bias_mu,
                "bias_rho": bias_rho,
                "bias_noise": bias_noise,
                "ard_alpha": np.ascontiguousarray(ard_alpha[isl]),
                "ard_beta": np.ascontiguousarray(ard_beta[isl]),
            }
        )

    res = run_bass_kernel_spmd(nc, in_maps, core_ids=list(range(NCORES)))
    LAST_RESULTS = res
    outs = res.results

    output = np.concatenate([outs[r]["out"] for r in range(NCORES)], axis=0)

    s = np.stack([outs[r]["sums"][0].astype(np.float64) for r in range(NCORES)])
    weight_kl = 0.5 * (2.0 * s[:, 0].sum() + s[:, 1].sum() + s[:, 2].sum()) - 0.5 * (
        OUT_F * IN_F
    )
    ard_kl = (s[:, 3] - s[:, 4] - s[:, 5]).sum()
    bias_kl = 0.5 * (2.0 * s[0, 6] + s[0, 7] + s[0, 8]) - 0.5 * OUT_F
    kl = np.float32(weight_kl + ard_kl + bias_kl)

    return output, kl
